# revision 12
# baseline (speedup 1.0000x reference)
"""DBRX-experts MoE kernel for 8 Trainium2 NeuronCores (expert-parallel).

Strategy
--------
E=8 experts map 1:1 onto the 8 cores. The host gathers each expert's routed
tokens (top-k dispatch done in numpy — the "all-to-all" of the sharding hint
collapses to a host-side gather because kernel() already owns the full
inputs), pads them to a common count, and pre-lays-out the expert's weights
so the device kernel is a pure dense transposed MLP:

    G^T = Wg^T-tiles @ X^T      (contract H, out [F, T])
    U^T = Wu^T-tiles @ X^T
    Hmid^T = sigmoid(G^T) * G^T * U^T        (silu(g) * u)
    Y^T = Wd^T-tiles @ Hmid^T   (contract F, out [H, T])

All matmuls keep the weights stationary ([128,128] tiles) and the tokens
moving ([128, <=512]); operands are bitcast to float32r so the PE runs at
1 cycle/row. The per-token combine weights and the scatter-add back into
the [T, H] output (the "all-reduce") are applied on the host.

No device collectives are needed: cores are fully independent.
"""

import os

# The axon jax platform must stay visible even if the caller pinned cpu for
# its own reference computation (bass2jax needs jax.devices() -> axon).
if os.environ.get("JAX_PLATFORMS") == "cpu":
    os.environ["JAX_PLATFORMS"] = ""

import numpy as np

import concourse.bass as bass
import concourse.mybir as mybir
import concourse.tile as tile
from concourse.bass_utils import run_bass_kernel_spmd

E, H, F, P = 8, 2048, 2048, 128
HO, FO = H // P, F // P  # 16, 16

F32 = mybir.dt.float32
F32R = mybir.dt.float32r
BF16 = mybir.dt.bfloat16

_prog_cache: dict = {}


def _chunks_for(n_pad: int):
    """Split [0, n_pad) into equal chunks of <=512 (one PSUM bank of fp32).

    Chunks should be >=256 where possible so float32r matmuls run at full
    rate (1 cycle/row needs moving free dim >= 256).
    """
    n_ch = -(-n_pad // 512)
    assert n_pad % n_ch == 0
    cn = n_pad // n_ch
    return [(i * cn, cn) for i in range(n_ch)]


def _pad_count(maxc: int) -> int:
    """Smallest padded token count: multiple of 256 >= maxc (min 512), so
    equal chunking into <=512-wide pieces keeps every chunk >=256 wide."""
    n = max(512, -(-maxc // 256) * 256)
    # make equal chunks divide evenly: n is a multiple of 256 and
    # ceil(n/512) chunks of n/ceil divide n when n/256 splits evenly.
    while n % (-(-n // 512)) != 0 or (n // (-(-n // 512))) % 2 != 0:
        n += 256
    return n


def _legalize_sync_waits(nc):
    """Split sync waits exceeding the per-instruction ISA budget into NOPs.

    This walrus build rejects instructions with too many embedded sync-wait
    commands ("Too many sync wait commands", CoreV3GenImpl setupSyncWait):
    Matmult (fp32r, self-loading weights) tolerates 1, most opcodes 2, and
    Tile's scheduler freely emits more (e.g. the kernel-tail Drain). Moving
    the excess waits onto NoOp instructions placed immediately before the
    offender on the same engine queue is semantically identical: the engine
    blocks on the NOP first, then issues the original instruction.
    """
    ctr = 0
    for fn in nc.m.functions:
        for blk in fn.blocks:
            insts = blk.instructions
            out = []
            changed = False
            for inst in insts:
                si = inst.sync_info
                waits = list(si.on_wait) if si is not None and si.on_wait else []
                limit = 1
                if len(waits) > limit:
                    extra, keep = waits[:-limit], waits[-limit:]
                    for w in extra:
                        nop = mybir.InstNoOp(name=f"ant_sync_split_{ctr}", ins=[], outs=[])
                        ctr += 1
                        nop.engine = inst.engine
                        nop.sync_info = mybir.SyncInfo(on_wait=[w], on_update=[])
                        out.append(nop)
                    si.on_wait = keep
                    changed = True
                out.append(inst)
            if changed:
                blk.instructions = out


def _build_program(n_pad: int, use_bf16: bool, legalize: bool = True):
    # fp32r operands must be *produced* as fp32r (BIR verifier: "consumed by
    # FP32r matmult but is not rounded to FP32r"), so the DRAM tensors and
    # every SBUF tile feeding a matmul are declared float32r end-to-end.
    dt_in = BF16 if use_bf16 else F32R
    chunks = _chunks_for(n_pad)

    nc = bass.Bass("TRN2")

    xt_d = nc.dram_tensor("xt", [P, HO, n_pad], dt_in, kind="ExternalInput")
    # gate and up interleaved on the second-to-last axis: one DMA per fo
    wgu_d = nc.dram_tensor("wgu", [FO, P, HO, 2, P], dt_in, kind="ExternalInput")
    wd_d = nc.dram_tensor("wd", [HO, P, FO, P], dt_in, kind="ExternalInput")
    yt_d = nc.dram_tensor("yt", [P, HO, n_pad], F32, kind="ExternalOutput")

    with tile.TileContext(nc) as tc:
        with (
            tc.tile_pool(name="xpool", bufs=1) as xpool,
            tc.tile_pool(name="wpool", bufs=2) as wpool,
            tc.tile_pool(name="hpool", bufs=1) as hpool,
            tc.tile_pool(name="tpool", bufs=3) as tpool,
            tc.tile_pool(name="pp", bufs=2, space="PSUM") as pp,
        ):
            # X^T resident in SBUF: [hi, ho, t]; one DMA per ho-slab so the
            # first matmuls can start before the whole tensor lands.
            xt = xpool.tile([P, HO, n_pad], dt_in)
            for ho in range(HO):
                nc.sync.dma_start(xt[:, ho], xt_d[:, ho])

            # Hmid^T resident in SBUF: [fi, fo, t]
            hmid = hpool.tile([P, FO, n_pad], dt_in)

            # Phase 1: G^T/U^T per 128-row slab of F, fused silu*up -> hmid
            for fo in range(FO):
                wgu_t = wpool.tile([P, HO, 2, P], dt_in, tag="wgu")
                nc.gpsimd.dma_start(wgu_t[:], wgu_d[fo])
                for c0, cn in chunks:
                    pg = pp.tile([P, cn], F32, tag="pg")
                    pu = pp.tile([P, cn], F32, tag="pu")
                    for ho in range(HO):
                        nc.tensor.matmul(
                            pg,
                            wgu_t[:, ho, 0],
                            xt[:, ho, c0 : c0 + cn],
                            start=ho == 0,
                            stop=ho == HO - 1,
                        )
                    for ho in range(HO):
                        nc.tensor.matmul(
                            pu,
                            wgu_t[:, ho, 1],
                            xt[:, ho, c0 : c0 + cn],
                            start=ho == 0,
                            stop=ho == HO - 1,
                        )
                    # silu(g) * u = sigmoid(g) * g * u
                    sg = tpool.tile([P, cn], F32, tag="sg")
                    nc.scalar.activation(
                        sg, pg, mybir.ActivationFunctionType.Sigmoid
                    )
                    gs = tpool.tile([P, cn], F32, tag="gs")
                    nc.vector.tensor_mul(out=gs, in0=sg, in1=pg)
                    nc.vector.tensor_mul(
                        out=hmid[:, fo, c0 : c0 + cn], in0=gs, in1=pu
                    )

            # Phase 2: Y^T per 128-row slab of H
            for ho in range(HO):
                wd_t = wpool.tile([P, FO, P], dt_in, tag="wd")
                nc.gpsimd.dma_start(wd_t[:], wd_d[ho])
                for c0, cn in chunks:
                    py = pp.tile([P, cn], F32, tag="py")
                    for fo in range(FO):
                        nc.tensor.matmul(
                            py,
                            wd_t[:, fo],
                            hmid[:, fo, c0 : c0 + cn],
                            start=fo == 0,
                            stop=fo == FO - 1,
                        )
                    yo = tpool.tile([P, cn], F32, tag="yo")
                    nc.vector.tensor_copy(out=yo, in_=py)
                    nc.sync.dma_start(yt_d[:, ho, c0 : c0 + cn], yo)

    if legalize:
        _legalize_sync_waits(nc)
    return nc


def _get_program(n_pad: int, use_bf16: bool, legalize: bool = True):
    key = (n_pad, use_bf16, legalize)
    if key not in _prog_cache:
        _prog_cache[key] = _build_program(n_pad, use_bf16, legalize)
    return _prog_cache[key]


def _route(top_experts: np.ndarray, top_weights: np.ndarray):
    """Per-expert token indices and combined weights (duplicates merged)."""
    te = np.asarray(top_experts).astype(np.int64)
    tw = np.asarray(top_weights, dtype=np.float32)
    idx_list, w_list = [], []
    for e in range(E):
        m = te == e
        sel = m.any(axis=1)
        idx = np.nonzero(sel)[0]
        w = (tw * m).sum(axis=1)[idx].astype(np.float32)
        idx_list.append(idx)
        w_list.append(w)
    return idx_list, w_list


def _np_dt(use_bf16: bool):
    if use_bf16:
        import ml_dtypes

        return np.dtype(ml_dtypes.bfloat16)
    return np.dtype(np.float32)


def _make_in_map(x, w_gate_e, w_up_e, w_down_e, idx, n_pad, use_bf16):
    npdt = _np_dt(use_bf16)
    n_e = len(idx)
    xt = np.zeros((H, n_pad), np.float32)
    if n_e:
        xt[:, :n_e] = x[idx].T
    # [h, t] -> [hi, ho, t]
    xt_dev = np.ascontiguousarray(
        xt.reshape(HO, P, n_pad).transpose(1, 0, 2)
    ).astype(npdt)
    # Wg[f, h] -> [fo, hi, ho, fi]  (lhsT tiles [hi, fi] for each (fo, ho));
    # gate and up stacked on a new axis -> [fo, hi, ho, 2, fi]
    wg_dev = (
        np.asarray(w_gate_e, np.float32).reshape(FO, P, HO, P).transpose(0, 3, 2, 1)
    )
    wu_dev = (
        np.asarray(w_up_e, np.float32).reshape(FO, P, HO, P).transpose(0, 3, 2, 1)
    )
    wgu_dev = np.ascontiguousarray(
        np.stack([wg_dev, wu_dev], axis=3)
    ).astype(npdt)
    # Wd[h, f] -> [ho, fi, fo, hi]  (lhsT tiles [fi, hi] for each (ho, fo))
    wd_dev = np.ascontiguousarray(
        np.asarray(w_down_e, np.float32)
        .reshape(HO, P, FO, P)
        .transpose(0, 3, 2, 1)
    ).astype(npdt)
    return {"xt": xt_dev, "wgu": wgu_dev, "wd": wd_dev}


def run(
    hidden_states,
    top_weights,
    w_gate,
    w_up,
    w_down,
    top_experts,
    use_bf16: bool = False,
    **spmd_kwargs,
):
    """Full MoE forward. Returns (output, BassKernelResults)."""
    x = np.asarray(hidden_states, dtype=np.float32).reshape(-1, H)
    T = x.shape[0]

    idx_list, w_list = _route(top_experts, top_weights)
    maxc = max(len(i) for i in idx_list)
    n_pad = _pad_count(maxc)

    nc = _get_program(n_pad, use_bf16)

    in_maps = [
        _make_in_map(
            x, w_gate[e], w_up[e], w_down[e], idx_list[e], n_pad, use_bf16
        )
        for e in range(E)
    ]

    res = run_bass_kernel_spmd(nc, in_maps, core_ids=list(range(E)), **spmd_kwargs)

    out = np.zeros((T, H), np.float32)
    for e in range(E):
        idx = idx_list[e]
        if len(idx) == 0:
            continue
        yt = res.results[e]["yt"]  # [hi, ho, t]
        y = yt.transpose(1, 0, 2).reshape(H, n_pad)[:, : len(idx)]  # [H, n_e]
        out[idx] += w_list[e][:, None] * y.T
    return out.reshape(np.asarray(hidden_states).shape).astype(np.float32), res


def kernel(hidden_states, top_weights, w_gate, w_up, w_down, top_experts):
    out, _ = run(hidden_states, top_weights, w_gate, w_up, w_down, top_experts)
    return out


# revision 16
# speedup vs baseline: 3.8260x; 3.8260x over previous
"""DBRX-experts MoE kernel for 8 Trainium2 NeuronCores (expert-parallel).

Strategy
--------
E=8 experts map 1:1 onto the 8 cores. The host gathers each expert's routed
tokens (top-k dispatch done in numpy — the "all-to-all" of the sharding hint
collapses to a host-side gather because kernel() already owns the full
inputs), pads them to a common count, and pre-lays-out the expert's weights
so the device kernel is a pure dense transposed MLP:

    G^T = Wg^T-tiles @ X^T      (contract H, out [F, T])
    U^T = Wu^T-tiles @ X^T
    Hmid^T = sigmoid(G^T) * G^T * U^T        (silu(g) * u)
    Y^T = Wd^T-tiles @ Hmid^T   (contract F, out [H, T])

All matmuls keep the weights stationary ([128,128] tiles) and the tokens
moving ([128, <=512]); operands are bitcast to float32r so the PE runs at
1 cycle/row. The per-token combine weights and the scatter-add back into
the [T, H] output (the "all-reduce") are applied on the host.

No device collectives are needed: cores are fully independent.
"""

import os

# The axon jax platform must stay visible even if the caller pinned cpu for
# its own reference computation (bass2jax needs jax.devices() -> axon).
if os.environ.get("JAX_PLATFORMS") == "cpu":
    os.environ["JAX_PLATFORMS"] = ""

import numpy as np

import concourse.bass as bass
import concourse.mybir as mybir
import concourse.tile as tile
from concourse.bass_utils import run_bass_kernel_spmd

E, H, F, P = 8, 2048, 2048, 128
HO, FO = H // P, F // P  # 16, 16

F32 = mybir.dt.float32
F32R = mybir.dt.float32r
BF16 = mybir.dt.bfloat16

_prog_cache: dict = {}


def _chunks_for(n_pad: int):
    """Split [0, n_pad) into equal chunks of <=512 (one PSUM bank of fp32).

    Chunks should be >=256 where possible so float32r matmuls run at full
    rate (1 cycle/row needs moving free dim >= 256).
    """
    n_ch = -(-n_pad // 512)
    assert n_pad % n_ch == 0
    cn = n_pad // n_ch
    return [(i * cn, cn) for i in range(n_ch)]


def _pad_count(maxc: int) -> int:
    """Smallest padded token count: multiple of 256 >= maxc (min 512), so
    equal chunking into <=512-wide pieces keeps every chunk >=256 wide."""
    n = max(512, -(-maxc // 256) * 256)
    # make equal chunks divide evenly: n is a multiple of 256 and
    # ceil(n/512) chunks of n/ceil divide n when n/256 splits evenly.
    while n % (-(-n // 512)) != 0 or (n // (-(-n // 512))) % 2 != 0:
        n += 256
    return n


def _legalize_sync_waits(nc):
    """Split sync waits exceeding the per-instruction ISA budget into NOPs.

    This walrus build rejects instructions with too many embedded sync-wait
    commands ("Too many sync wait commands", CoreV3GenImpl setupSyncWait):
    Matmult (fp32r, self-loading weights) tolerates 1, most opcodes 2, and
    Tile's scheduler freely emits more (e.g. the kernel-tail Drain). Moving
    the excess waits onto NoOp instructions placed immediately before the
    offender on the same engine queue is semantically identical: the engine
    blocks on the NOP first, then issues the original instruction.
    """
    ctr = 0
    for fn in nc.m.functions:
        for blk in fn.blocks:
            insts = blk.instructions
            out = []
            changed = False
            for inst in insts:
                si = inst.sync_info
                waits = list(si.on_wait) if si is not None and si.on_wait else []
                limit = 1
                if len(waits) > limit:
                    extra, keep = waits[:-limit], waits[-limit:]
                    for w in extra:
                        nop = mybir.InstNoOp(name=f"ant_sync_split_{ctr}", ins=[], outs=[])
                        ctr += 1
                        nop.engine = inst.engine
                        nop.sync_info = mybir.SyncInfo(on_wait=[w], on_update=[])
                        out.append(nop)
                    si.on_wait = keep
                    changed = True
                out.append(inst)
            if changed:
                blk.instructions = out


def _build_program(
    n_pad: int, use_bf16: bool, legalize: bool = True, reps: int = 1
):
    # fp32r operands must be *produced* as fp32r (BIR verifier: "consumed by
    # FP32r matmult but is not rounded to FP32r"), so the DRAM tensors and
    # every SBUF tile feeding a matmul are declared float32r end-to-end.
    dt_in = BF16 if use_bf16 else F32R
    chunks = _chunks_for(n_pad)

    nc = bass.Bass("TRN2")

    xt_d = nc.dram_tensor("xt", [P, HO, n_pad], dt_in, kind="ExternalInput")
    # gate and up interleaved on the second-to-last axis: one DMA per fo
    wgu_d = nc.dram_tensor("wgu", [FO, P, HO, 2, P], dt_in, kind="ExternalInput")
    wd_d = nc.dram_tensor("wd", [HO, P, FO, P], dt_in, kind="ExternalInput")
    yt_d = nc.dram_tensor("yt", [P, HO, n_pad], F32, kind="ExternalOutput")

    with tile.TileContext(nc) as tc:
        with (
            tc.tile_pool(name="xpool", bufs=1) as xpool,
            tc.tile_pool(name="wpool", bufs=2) as wpool,
            tc.tile_pool(name="hpool", bufs=1) as hpool,
            tc.tile_pool(name="tpool", bufs=3) as tpool,
            tc.tile_pool(name="pp", bufs=2, space="PSUM") as pp,
        ):
            # X^T resident in SBUF: [hi, ho, t]; one DMA per ho-slab so the
            # first matmuls can start before the whole tensor lands.
            xt = xpool.tile([P, HO, n_pad], dt_in)
            for ho in range(HO):
                nc.sync.dma_start(xt[:, ho], xt_d[:, ho])

            for _rep in range(reps):
                _emit_mlp_body(
                    nc, n_pad, chunks, dt_in, xt,
                    wgu_d, wd_d, yt_d, wpool, hpool, tpool, pp,
                )

    if legalize:
        _legalize_sync_waits(nc)
    return nc


def _emit_mlp_body(
    nc, n_pad, chunks, dt_in, xt, wgu_d, wd_d, yt_d, wpool, hpool, tpool, pp
):
    P_, HO_, FO_ = P, HO, FO
    if True:
        if True:
            # Hmid^T resident in SBUF: [fi, fo, t]
            hmid = hpool.tile([P, FO, n_pad], dt_in, tag="hmid")

            # Phase 1: G^T/U^T per 128-row slab of F, fused silu*up -> hmid
            for fo in range(FO):
                wgu_t = wpool.tile([P, HO, 2, P], dt_in, tag="wgu")
                nc.gpsimd.dma_start(wgu_t[:], wgu_d[fo])
                for c0, cn in chunks:
                    pg = pp.tile([P, cn], F32, tag="pg")
                    pu = pp.tile([P, cn], F32, tag="pu")
                    for ho in range(HO):
                        nc.tensor.matmul(
                            pg,
                            wgu_t[:, ho, 0],
                            xt[:, ho, c0 : c0 + cn],
                            start=ho == 0,
                            stop=ho == HO - 1,
                        )
                    for ho in range(HO):
                        nc.tensor.matmul(
                            pu,
                            wgu_t[:, ho, 1],
                            xt[:, ho, c0 : c0 + cn],
                            start=ho == 0,
                            stop=ho == HO - 1,
                        )
                    # silu(g) * u = sigmoid(g) * g * u
                    sg = tpool.tile([P, cn], F32, tag="sg")
                    nc.scalar.activation(
                        sg, pg, mybir.ActivationFunctionType.Sigmoid
                    )
                    gs = tpool.tile([P, cn], F32, tag="gs")
                    nc.vector.tensor_mul(out=gs, in0=sg, in1=pg)
                    nc.vector.tensor_mul(
                        out=hmid[:, fo, c0 : c0 + cn], in0=gs, in1=pu
                    )

            # Phase 2: Y^T per 128-row slab of H
            for ho in range(HO):
                wd_t = wpool.tile([P, FO, P], dt_in, tag="wd")
                nc.gpsimd.dma_start(wd_t[:], wd_d[ho])
                for c0, cn in chunks:
                    py = pp.tile([P, cn], F32, tag="py")
                    for fo in range(FO):
                        nc.tensor.matmul(
                            py,
                            wd_t[:, fo],
                            hmid[:, fo, c0 : c0 + cn],
                            start=fo == 0,
                            stop=fo == FO - 1,
                        )
                    yo = tpool.tile([P, cn], F32, tag="yo")
                    nc.vector.tensor_copy(out=yo, in_=py)
                    nc.sync.dma_start(yt_d[:, ho, c0 : c0 + cn], yo)


def _get_program(
    n_pad: int, use_bf16: bool, legalize: bool = True, reps: int = 1
):
    key = (n_pad, use_bf16, legalize, reps)
    if key not in _prog_cache:
        _prog_cache[key] = _build_program(n_pad, use_bf16, legalize, reps)
    return _prog_cache[key]


def _route(top_experts: np.ndarray, top_weights: np.ndarray):
    """Per-expert token indices and combined weights (duplicates merged)."""
    te = np.asarray(top_experts).astype(np.int64)
    tw = np.asarray(top_weights, dtype=np.float32)
    idx_list, w_list = [], []
    for e in range(E):
        m = te == e
        sel = m.any(axis=1)
        idx = np.nonzero(sel)[0]
        w = (tw * m).sum(axis=1)[idx].astype(np.float32)
        idx_list.append(idx)
        w_list.append(w)
    return idx_list, w_list


def _np_dt(use_bf16: bool):
    if use_bf16:
        import ml_dtypes

        return np.dtype(ml_dtypes.bfloat16)
    return np.dtype(np.float32)


def _make_in_map(x, w_gate_e, w_up_e, w_down_e, idx, n_pad, use_bf16):
    npdt = _np_dt(use_bf16)
    n_e = len(idx)
    xt = np.zeros((H, n_pad), np.float32)
    if n_e:
        xt[:, :n_e] = x[idx].T
    # [h, t] -> [hi, ho, t]
    xt_dev = np.ascontiguousarray(
        xt.reshape(HO, P, n_pad).transpose(1, 0, 2)
    ).astype(npdt)
    # Wg[f, h] -> [fo, hi, ho, fi]  (lhsT tiles [hi, fi] for each (fo, ho));
    # gate and up stacked on a new axis -> [fo, hi, ho, 2, fi]
    wg_dev = (
        np.asarray(w_gate_e, np.float32).reshape(FO, P, HO, P).transpose(0, 3, 2, 1)
    )
    wu_dev = (
        np.asarray(w_up_e, np.float32).reshape(FO, P, HO, P).transpose(0, 3, 2, 1)
    )
    wgu_dev = np.ascontiguousarray(
        np.stack([wg_dev, wu_dev], axis=3)
    ).astype(npdt)
    # Wd[h, f] -> [ho, fi, fo, hi]  (lhsT tiles [fi, hi] for each (ho, fo))
    wd_dev = np.ascontiguousarray(
        np.asarray(w_down_e, np.float32)
        .reshape(HO, P, FO, P)
        .transpose(0, 3, 2, 1)
    ).astype(npdt)
    return {"xt": xt_dev, "wgu": wgu_dev, "wd": wd_dev}


def run(
    hidden_states,
    top_weights,
    w_gate,
    w_up,
    w_down,
    top_experts,
    use_bf16: bool = False,
    **spmd_kwargs,
):
    """Full MoE forward. Returns (output, BassKernelResults)."""
    x = np.asarray(hidden_states, dtype=np.float32).reshape(-1, H)
    T = x.shape[0]

    idx_list, w_list = _route(top_experts, top_weights)
    maxc = max(len(i) for i in idx_list)
    n_pad = _pad_count(maxc)

    nc = _get_program(n_pad, use_bf16)

    in_maps = [
        _make_in_map(
            x, w_gate[e], w_up[e], w_down[e], idx_list[e], n_pad, use_bf16
        )
        for e in range(E)
    ]

    try:
        res = run_bass_kernel_spmd(
            nc, in_maps, core_ids=list(range(E)), **spmd_kwargs
        )
    except Exception:
        # Transient NRT exec failures have been observed on the first
        # 8-core execution after environment start; one retry clears them.
        res = run_bass_kernel_spmd(
            nc, in_maps, core_ids=list(range(E)), **spmd_kwargs
        )

    out = np.zeros((T, H), np.float32)
    for e in range(E):
        idx = idx_list[e]
        if len(idx) == 0:
            continue
        yt = res.results[e]["yt"]  # [hi, ho, t]
        y = yt.transpose(1, 0, 2).reshape(H, n_pad)[:, : len(idx)]  # [H, n_e]
        out[idx] += w_list[e][:, None] * y.T
    return out.reshape(np.asarray(hidden_states).shape).astype(np.float32), res


def kernel(hidden_states, top_weights, w_gate, w_up, w_down, top_experts):
    out, _ = run(hidden_states, top_weights, w_gate, w_up, w_down, top_experts)
    return out


# revision 24
# speedup vs baseline: 4.6685x; 1.2202x over previous
"""DBRX-experts MoE kernel for 8 Trainium2 NeuronCores (expert-parallel).

Strategy
--------
E=8 experts map 1:1 onto the 8 cores. The host gathers each expert's routed
tokens (top-k dispatch done in numpy — the "all-to-all" of the sharding hint
collapses to a host-side gather because kernel() already owns the full
inputs), pads them to a common count, and pre-lays-out the expert's weights
so the device kernel is a pure dense transposed MLP:

    G^T = Wg^T-tiles @ X^T      (contract H, out [F, T])
    U^T = Wu^T-tiles @ X^T
    Hmid^T = sigmoid(G^T) * G^T * U^T        (silu(g) * u)
    Y^T = Wd^T-tiles @ Hmid^T   (contract F, out [H, T])

All matmuls keep the weights stationary ([128,128] tiles) and the tokens
moving ([128, <=512]); operands are bitcast to float32r so the PE runs at
1 cycle/row. The per-token combine weights and the scatter-add back into
the [T, H] output (the "all-reduce") are applied on the host.

No device collectives are needed: cores are fully independent.
"""

import os

# The axon jax platform must stay visible even if the caller pinned cpu for
# its own reference computation (bass2jax needs jax.devices() -> axon).
if os.environ.get("JAX_PLATFORMS") == "cpu":
    os.environ["JAX_PLATFORMS"] = ""

import numpy as np

import concourse.bass as bass
import concourse.mybir as mybir
import concourse.tile as tile
from concourse.bass_utils import run_bass_kernel_spmd

E, H, F, P = 8, 2048, 2048, 128
HO, FO = H // P, F // P  # 16, 16

F32 = mybir.dt.float32
F32R = mybir.dt.float32r
BF16 = mybir.dt.bfloat16

_prog_cache: dict = {}


def _chunks_for(n_pad: int):
    """Split [0, n_pad) into equal chunks of <=512 (one PSUM bank of fp32).

    Chunks should be >=256 where possible so float32r matmuls run at full
    rate (1 cycle/row needs moving free dim >= 256).
    """
    n_ch = -(-n_pad // 512)
    assert n_pad % n_ch == 0
    cn = n_pad // n_ch
    return [(i * cn, cn) for i in range(n_ch)]


def _pad_count(maxc: int) -> int:
    """Smallest padded token count: multiple of 256 >= maxc (min 512), so
    equal chunking into <=512-wide pieces keeps every chunk >=256 wide."""
    n = max(512, -(-maxc // 256) * 256)
    # make equal chunks divide evenly: n is a multiple of 256 and
    # ceil(n/512) chunks of n/ceil divide n when n/256 splits evenly.
    while n % (-(-n // 512)) != 0 or (n // (-(-n // 512))) % 2 != 0:
        n += 256
    return n


def _legalize_sync_waits(nc):
    """Split sync waits exceeding the per-instruction ISA budget into NOPs.

    This walrus build rejects instructions with too many embedded sync-wait
    commands ("Too many sync wait commands", CoreV3GenImpl setupSyncWait):
    Matmult (fp32r, self-loading weights) tolerates 1, most opcodes 2, and
    Tile's scheduler freely emits more (e.g. the kernel-tail Drain). Moving
    the excess waits onto NoOp instructions placed immediately before the
    offender on the same engine queue is semantically identical: the engine
    blocks on the NOP first, then issues the original instruction.
    """
    ctr = 0
    for fn in nc.m.functions:
        for blk in fn.blocks:
            insts = blk.instructions
            out = []
            changed = False
            for inst in insts:
                si = inst.sync_info
                waits = list(si.on_wait) if si is not None and si.on_wait else []
                limit = 1
                if len(waits) > limit:
                    extra, keep = waits[:-limit], waits[-limit:]
                    for w in extra:
                        nop = mybir.InstNoOp(name=f"ant_sync_split_{ctr}", ins=[], outs=[])
                        ctr += 1
                        nop.engine = inst.engine
                        nop.sync_info = mybir.SyncInfo(on_wait=[w], on_update=[])
                        out.append(nop)
                    si.on_wait = keep
                    changed = True
                out.append(inst)
            if changed:
                blk.instructions = out


def _build_program(
    n_pad: int,
    use_bf16: bool,
    legalize: bool = True,
    reps: int = 1,
    wpool_bufs: int = 3,
    pp_bufs: int = 2,
    tpool_bufs: int = 3,
    gu_bufs: int | None = None,
    py_bufs: int | None = None,
    w_engine: str = "gpsimd",
):
    # fp32r operands must be *produced* as fp32r (BIR verifier: "consumed by
    # FP32r matmult but is not rounded to FP32r"), so the DRAM tensors and
    # every SBUF tile feeding a matmul are declared float32r end-to-end.
    dt_in = BF16 if use_bf16 else F32R
    chunks = _chunks_for(n_pad)

    nc = bass.Bass("TRN2")

    xt_d = nc.dram_tensor("xt", [P, HO, n_pad], dt_in, kind="ExternalInput")
    # gate and up interleaved on the second-to-last axis: one DMA per fo
    wgu_d = nc.dram_tensor("wgu", [FO, P, HO, 2, P], dt_in, kind="ExternalInput")
    wd_d = nc.dram_tensor("wd", [HO, P, FO, P], dt_in, kind="ExternalInput")
    yt_d = nc.dram_tensor("yt", [P, HO, n_pad], F32, kind="ExternalOutput")

    with tile.TileContext(nc) as tc:
        with (
            tc.tile_pool(name="xpool", bufs=1) as xpool,
            tc.tile_pool(name="wpool", bufs=wpool_bufs) as wpool,
            tc.tile_pool(name="hpool", bufs=1) as hpool,
            tc.tile_pool(name="tpool", bufs=tpool_bufs) as tpool,
            tc.tile_pool(name="pp", bufs=pp_bufs, space="PSUM") as pp,
        ):
            # X^T resident in SBUF: [hi, ho, t]; one DMA per ho-slab so the
            # first matmuls can start before the whole tensor lands.
            xt = xpool.tile([P, HO, n_pad], dt_in)
            for ho in range(HO):
                nc.sync.dma_start(xt[:, ho], xt_d[:, ho])

            for _rep in range(reps):
                _emit_mlp_body(
                    nc, n_pad, chunks, dt_in, xt,
                    wgu_d, wd_d, yt_d, wpool, hpool, tpool, pp,
                    gu_bufs=gu_bufs, py_bufs=py_bufs, w_engine=w_engine,
                )

    if legalize:
        _legalize_sync_waits(nc)
    return nc


def _emit_mlp_body(
    nc, n_pad, chunks, dt_in, xt, wgu_d, wd_d, yt_d, wpool, hpool, tpool, pp,
    gu_bufs=None, py_bufs=None, w_engine="gpsimd",
):
    w_dma = nc.gpsimd.dma_start if w_engine == "gpsimd" else nc.sync.dma_start
    if True:
        if True:
            # Hmid^T resident in SBUF: [fi, fo, t]
            hmid = hpool.tile([P, FO, n_pad], dt_in, tag="hmid")

            # Phase 1: G^T/U^T per 128-row slab of F, fused silu*up -> hmid
            for fo in range(FO):
                wgu_t = wpool.tile([P, HO, 2, P], dt_in, tag="wgu")
                w_dma(wgu_t[:], wgu_d[fo])
                for c0, cn in chunks:
                    pg = pp.tile([P, cn], F32, tag="pg", bufs=gu_bufs)
                    pu = pp.tile([P, cn], F32, tag="pu", bufs=gu_bufs)
                    for ho in range(HO):
                        nc.tensor.matmul(
                            pg,
                            wgu_t[:, ho, 0],
                            xt[:, ho, c0 : c0 + cn],
                            start=ho == 0,
                            stop=ho == HO - 1,
                        )
                    for ho in range(HO):
                        nc.tensor.matmul(
                            pu,
                            wgu_t[:, ho, 1],
                            xt[:, ho, c0 : c0 + cn],
                            start=ho == 0,
                            stop=ho == HO - 1,
                        )
                    # silu(g) * u = sigmoid(g) * g * u
                    sg = tpool.tile([P, cn], F32, tag="sg")
                    nc.scalar.activation(
                        sg, pg, mybir.ActivationFunctionType.Sigmoid
                    )
                    gs = tpool.tile([P, cn], F32, tag="gs")
                    nc.vector.tensor_mul(out=gs, in0=sg, in1=pg)
                    nc.vector.tensor_mul(
                        out=hmid[:, fo, c0 : c0 + cn], in0=gs, in1=pu
                    )

            # Phase 2: Y^T per 128-row slab of H
            for ho in range(HO):
                wd_t = wpool.tile([P, FO, P], dt_in, tag="wd")
                w_dma(wd_t[:], wd_d[ho])
                for c0, cn in chunks:
                    py = pp.tile([P, cn], F32, tag="py", bufs=py_bufs)
                    for fo in range(FO):
                        nc.tensor.matmul(
                            py,
                            wd_t[:, fo],
                            hmid[:, fo, c0 : c0 + cn],
                            start=fo == 0,
                            stop=fo == FO - 1,
                        )
                    yo = tpool.tile([P, cn], F32, tag="yo")
                    nc.vector.tensor_copy(out=yo, in_=py)
                    nc.sync.dma_start(yt_d[:, ho, c0 : c0 + cn], yo)


def _get_program(
    n_pad: int, use_bf16: bool, legalize: bool = True, reps: int = 1, **kw
):
    key = (n_pad, use_bf16, legalize, reps, tuple(sorted(kw.items())))
    if key not in _prog_cache:
        _prog_cache[key] = _build_program(n_pad, use_bf16, legalize, reps, **kw)
    return _prog_cache[key]


def _route(top_experts: np.ndarray, top_weights: np.ndarray):
    """Per-expert token indices and combined weights (duplicates merged)."""
    te = np.asarray(top_experts).astype(np.int64)
    tw = np.asarray(top_weights, dtype=np.float32)
    idx_list, w_list = [], []
    for e in range(E):
        m = te == e
        sel = m.any(axis=1)
        idx = np.nonzero(sel)[0]
        w = (tw * m).sum(axis=1)[idx].astype(np.float32)
        idx_list.append(idx)
        w_list.append(w)
    return idx_list, w_list


def _np_dt(use_bf16: bool):
    if use_bf16:
        import ml_dtypes

        return np.dtype(ml_dtypes.bfloat16)
    return np.dtype(np.float32)


def _make_in_map(x, w_gate_e, w_up_e, w_down_e, idx, n_pad, use_bf16):
    npdt = _np_dt(use_bf16)
    n_e = len(idx)
    xt = np.zeros((H, n_pad), np.float32)
    if n_e:
        xt[:, :n_e] = x[idx].T
    # [h, t] -> [hi, ho, t]
    xt_dev = np.ascontiguousarray(
        xt.reshape(HO, P, n_pad).transpose(1, 0, 2)
    ).astype(npdt)
    # Wg[f, h] -> [fo, hi, ho, fi]  (lhsT tiles [hi, fi] for each (fo, ho));
    # gate and up stacked on a new axis -> [fo, hi, ho, 2, fi]
    wg_dev = (
        np.asarray(w_gate_e, np.float32).reshape(FO, P, HO, P).transpose(0, 3, 2, 1)
    )
    wu_dev = (
        np.asarray(w_up_e, np.float32).reshape(FO, P, HO, P).transpose(0, 3, 2, 1)
    )
    wgu_dev = np.ascontiguousarray(
        np.stack([wg_dev, wu_dev], axis=3)
    ).astype(npdt)
    # Wd[h, f] -> [ho, fi, fo, hi]  (lhsT tiles [fi, hi] for each (ho, fo))
    wd_dev = np.ascontiguousarray(
        np.asarray(w_down_e, np.float32)
        .reshape(HO, P, FO, P)
        .transpose(0, 3, 2, 1)
    ).astype(npdt)
    return {"xt": xt_dev, "wgu": wgu_dev, "wd": wd_dev}


def run(
    hidden_states,
    top_weights,
    w_gate,
    w_up,
    w_down,
    top_experts,
    use_bf16: bool = False,
    **spmd_kwargs,
):
    """Full MoE forward. Returns (output, BassKernelResults)."""
    x = np.asarray(hidden_states, dtype=np.float32).reshape(-1, H)
    T = x.shape[0]

    idx_list, w_list = _route(top_experts, top_weights)
    maxc = max(len(i) for i in idx_list)
    n_pad = _pad_count(maxc)

    nc = _get_program(n_pad, use_bf16)

    in_maps = [
        _make_in_map(
            x, w_gate[e], w_up[e], w_down[e], idx_list[e], n_pad, use_bf16
        )
        for e in range(E)
    ]

    try:
        res = run_bass_kernel_spmd(
            nc, in_maps, core_ids=list(range(E)), **spmd_kwargs
        )
    except Exception:
        # Transient NRT exec failures have been observed on the first
        # 8-core execution after environment start; one retry clears them.
        res = run_bass_kernel_spmd(
            nc, in_maps, core_ids=list(range(E)), **spmd_kwargs
        )

    out = np.zeros((T, H), np.float32)
    for e in range(E):
        idx = idx_list[e]
        if len(idx) == 0:
            continue
        yt = res.results[e]["yt"]  # [hi, ho, t]
        y = yt.transpose(1, 0, 2).reshape(H, n_pad)[:, : len(idx)]  # [H, n_e]
        out[idx] += w_list[e][:, None] * y.T
    return out.reshape(np.asarray(hidden_states).shape).astype(np.float32), res


def kernel(hidden_states, top_weights, w_gate, w_up, w_down, top_experts):
    out, _ = run(hidden_states, top_weights, w_gate, w_up, w_down, top_experts)
    return out


# revision 26
# speedup vs baseline: 4.8900x; 1.0475x over previous
"""DBRX-experts MoE kernel for 8 Trainium2 NeuronCores (expert-parallel).

Strategy
--------
E=8 experts map 1:1 onto the 8 cores. The host gathers each expert's routed
tokens (top-k dispatch done in numpy — the "all-to-all" of the sharding hint
collapses to a host-side gather because kernel() already owns the full
inputs), pads them to a common count, and pre-lays-out the expert's weights
so the device kernel is a pure dense transposed MLP:

    G^T = Wg^T-tiles @ X^T      (contract H, out [F, T])
    U^T = Wu^T-tiles @ X^T
    Hmid^T = sigmoid(G^T) * G^T * U^T        (silu(g) * u)
    Y^T = Wd^T-tiles @ Hmid^T   (contract F, out [H, T])

All matmuls keep the weights stationary ([128,128] tiles) and the tokens
moving ([128, <=512]); tensors are declared float32r end-to-end so the PE
runs at 1 cycle/row (~2e-4 rel err vs fp32). The per-token combine weights
and the scatter-add back into the [T, H] output (the "all-reduce") are
applied on the host.

No device collectives are needed: cores are fully independent.
"""

import os

# The axon jax platform must stay visible even if the caller pinned cpu for
# its own reference computation (bass2jax needs jax.devices() -> axon).
if os.environ.get("JAX_PLATFORMS") == "cpu":
    os.environ["JAX_PLATFORMS"] = ""

import numpy as np

import concourse.bass as bass
import concourse.mybir as mybir
import concourse.tile as tile
from concourse.bass_utils import run_bass_kernel_spmd

E, H, F, P = 8, 2048, 2048, 128
HO, FO = H // P, F // P  # 16, 16

F32 = mybir.dt.float32
F32R = mybir.dt.float32r
BF16 = mybir.dt.bfloat16

_prog_cache: dict = {}


def _chunks_for(n_pad: int):
    """Split [0, n_pad) into equal chunks of <=512 (one PSUM bank of fp32).

    Chunks should be >=256 where possible so float32r matmuls run at full
    rate (1 cycle/row needs moving free dim >= 256).
    """
    n_ch = -(-n_pad // 512)
    assert n_pad % n_ch == 0
    cn = n_pad // n_ch
    return [(i * cn, cn) for i in range(n_ch)]


def _pad_count(maxc: int) -> int:
    """Smallest padded token count: multiple of 256 >= maxc (min 512), so
    equal chunking into <=512-wide pieces keeps every chunk >=256 wide."""
    n = max(512, -(-maxc // 256) * 256)
    # make equal chunks divide evenly: n is a multiple of 256 and
    # ceil(n/512) chunks of n/ceil divide n when n/256 splits evenly.
    while n % (-(-n // 512)) != 0 or (n // (-(-n // 512))) % 2 != 0:
        n += 256
    return n


def _legalize_sync_waits(nc):
    """Split sync waits exceeding the per-instruction ISA budget into NOPs.

    This walrus build rejects instructions with too many embedded sync-wait
    commands ("Too many sync wait commands", CoreV3GenImpl setupSyncWait):
    Matmult (fp32r, self-loading weights) tolerates 1, most opcodes 2, and
    Tile's scheduler freely emits more (e.g. the kernel-tail Drain). Moving
    the excess waits onto NoOp instructions placed immediately before the
    offender on the same engine queue is semantically identical: the engine
    blocks on the NOP first, then issues the original instruction.
    """
    ctr = 0
    for fn in nc.m.functions:
        for blk in fn.blocks:
            insts = blk.instructions
            out = []
            changed = False
            for inst in insts:
                si = inst.sync_info
                waits = list(si.on_wait) if si is not None and si.on_wait else []
                limit = 1
                if len(waits) > limit:
                    extra, keep = waits[:-limit], waits[-limit:]
                    for w in extra:
                        nop = mybir.InstNoOp(name=f"ant_sync_split_{ctr}", ins=[], outs=[])
                        ctr += 1
                        nop.engine = inst.engine
                        nop.sync_info = mybir.SyncInfo(on_wait=[w], on_update=[])
                        out.append(nop)
                    si.on_wait = keep
                    changed = True
                out.append(inst)
            if changed:
                blk.instructions = out


def _build_program(
    n_pad: int,
    use_bf16: bool,
    legalize: bool = True,
    reps: int = 1,
    wpool_bufs: int = 3,
    pp_bufs: int = 2,
    tpool_bufs: int = 3,
    gu_bufs: int | None = None,
    py_bufs: int | None = None,
    w_engine: str = "gpsimd",
):
    # fp32r operands must be *produced* as fp32r (BIR verifier: "consumed by
    # FP32r matmult but is not rounded to FP32r"), so the DRAM tensors and
    # every SBUF tile feeding a matmul are declared float32r end-to-end.
    dt_in = BF16 if use_bf16 else F32R
    chunks = _chunks_for(n_pad)

    nc = bass.Bass("TRN2")

    xt_d = nc.dram_tensor("xt", [P, HO, n_pad], dt_in, kind="ExternalInput")
    # gate and up interleaved on the second-to-last axis: one DMA per fo
    wgu_d = nc.dram_tensor("wgu", [FO, P, HO, 2, P], dt_in, kind="ExternalInput")
    wd_d = nc.dram_tensor("wd", [HO, P, FO, P], dt_in, kind="ExternalInput")
    yt_d = nc.dram_tensor("yt", [P, HO, n_pad], F32, kind="ExternalOutput")

    with tile.TileContext(nc) as tc:
        with (
            tc.tile_pool(name="xpool", bufs=1) as xpool,
            tc.tile_pool(name="wpool", bufs=wpool_bufs) as wpool,
            tc.tile_pool(name="hpool", bufs=1) as hpool,
            tc.tile_pool(name="tpool", bufs=tpool_bufs) as tpool,
            tc.tile_pool(name="pp", bufs=pp_bufs, space="PSUM") as pp,
        ):
            # X^T resident in SBUF: [hi, ho, t]; one DMA per ho-slab so the
            # first matmuls can start before the whole tensor lands.
            xt = xpool.tile([P, HO, n_pad], dt_in)
            for ho in range(HO):
                nc.sync.dma_start(xt[:, ho], xt_d[:, ho])

            for _rep in range(reps):
                _emit_mlp_body(
                    nc, n_pad, chunks, dt_in, xt,
                    wgu_d, wd_d, yt_d, wpool, hpool, tpool, pp,
                    gu_bufs=gu_bufs, py_bufs=py_bufs, w_engine=w_engine,
                )

    if legalize:
        _legalize_sync_waits(nc)
    return nc


def _emit_mlp_body(
    nc, n_pad, chunks, dt_in, xt, wgu_d, wd_d, yt_d, wpool, hpool, tpool, pp,
    gu_bufs=None, py_bufs=None, w_engine="gpsimd",
):
    w_dma = nc.gpsimd.dma_start if w_engine == "gpsimd" else nc.sync.dma_start
    if True:
        if True:
            # Hmid^T resident in SBUF: [fi, fo, t]
            hmid = hpool.tile([P, FO, n_pad], dt_in, tag="hmid")

            # Phase 1: G^T/U^T per 128-row slab of F, fused silu*up -> hmid
            for fo in range(FO):
                wgu_t = wpool.tile([P, HO, 2, P], dt_in, tag="wgu")
                w_dma(wgu_t[:], wgu_d[fo])
                for c0, cn in chunks:
                    pg = pp.tile([P, cn], F32, tag="pg", bufs=gu_bufs)
                    pu = pp.tile([P, cn], F32, tag="pu", bufs=gu_bufs)
                    for ho in range(HO):
                        nc.tensor.matmul(
                            pg,
                            wgu_t[:, ho, 0],
                            xt[:, ho, c0 : c0 + cn],
                            start=ho == 0,
                            stop=ho == HO - 1,
                        )
                    for ho in range(HO):
                        nc.tensor.matmul(
                            pu,
                            wgu_t[:, ho, 1],
                            xt[:, ho, c0 : c0 + cn],
                            start=ho == 0,
                            stop=ho == HO - 1,
                        )
                    # silu(g) * u = sigmoid(g) * g * u
                    sg = tpool.tile([P, cn], F32, tag="sg")
                    nc.scalar.activation(
                        sg, pg, mybir.ActivationFunctionType.Sigmoid
                    )
                    gs = tpool.tile([P, cn], F32, tag="gs")
                    nc.vector.tensor_mul(out=gs, in0=sg, in1=pg)
                    nc.vector.tensor_mul(
                        out=hmid[:, fo, c0 : c0 + cn], in0=gs, in1=pu
                    )

            # Phase 2: Y^T per 128-row slab of H
            for ho in range(HO):
                wd_t = wpool.tile([P, FO, P], dt_in, tag="wd")
                w_dma(wd_t[:], wd_d[ho])
                for c0, cn in chunks:
                    py = pp.tile([P, cn], F32, tag="py", bufs=py_bufs)
                    for fo in range(FO):
                        nc.tensor.matmul(
                            py,
                            wd_t[:, fo],
                            hmid[:, fo, c0 : c0 + cn],
                            start=fo == 0,
                            stop=fo == FO - 1,
                        )
                    yo = tpool.tile([P, cn], F32, tag="yo")
                    nc.vector.tensor_copy(out=yo, in_=py)
                    nc.sync.dma_start(yt_d[:, ho, c0 : c0 + cn], yo)


def _get_program(
    n_pad: int, use_bf16: bool, legalize: bool = True, reps: int = 1, **kw
):
    key = (n_pad, use_bf16, legalize, reps, tuple(sorted(kw.items())))
    if key not in _prog_cache:
        _prog_cache[key] = _build_program(n_pad, use_bf16, legalize, reps, **kw)
    return _prog_cache[key]


def _route(top_experts: np.ndarray, top_weights: np.ndarray):
    """Per-expert token indices and combined weights (duplicates merged)."""
    te = np.asarray(top_experts).astype(np.int64)
    tw = np.asarray(top_weights, dtype=np.float32)
    idx_list, w_list = [], []
    for e in range(E):
        m = te == e
        sel = m.any(axis=1)
        idx = np.nonzero(sel)[0]
        w = (tw * m).sum(axis=1)[idx].astype(np.float32)
        idx_list.append(idx)
        w_list.append(w)
    return idx_list, w_list


def _np_dt(use_bf16: bool):
    if use_bf16:
        import ml_dtypes

        return np.dtype(ml_dtypes.bfloat16)
    return np.dtype(np.float32)


def _make_in_map(x, w_gate_e, w_up_e, w_down_e, idx, n_pad, use_bf16):
    npdt = _np_dt(use_bf16)
    n_e = len(idx)
    xt = np.zeros((H, n_pad), np.float32)
    if n_e:
        xt[:, :n_e] = x[idx].T
    # [h, t] -> [hi, ho, t]
    xt_dev = np.ascontiguousarray(
        xt.reshape(HO, P, n_pad).transpose(1, 0, 2)
    ).astype(npdt)
    # Wg[f, h] -> [fo, hi, ho, fi]  (lhsT tiles [hi, fi] for each (fo, ho));
    # gate and up stacked on a new axis -> [fo, hi, ho, 2, fi]
    wg_dev = (
        np.asarray(w_gate_e, np.float32).reshape(FO, P, HO, P).transpose(0, 3, 2, 1)
    )
    wu_dev = (
        np.asarray(w_up_e, np.float32).reshape(FO, P, HO, P).transpose(0, 3, 2, 1)
    )
    wgu_dev = np.ascontiguousarray(
        np.stack([wg_dev, wu_dev], axis=3)
    ).astype(npdt)
    # Wd[h, f] -> [ho, fi, fo, hi]  (lhsT tiles [fi, hi] for each (ho, fo))
    wd_dev = np.ascontiguousarray(
        np.asarray(w_down_e, np.float32)
        .reshape(HO, P, FO, P)
        .transpose(0, 3, 2, 1)
    ).astype(npdt)
    return {"xt": xt_dev, "wgu": wgu_dev, "wd": wd_dev}


def run(
    hidden_states,
    top_weights,
    w_gate,
    w_up,
    w_down,
    top_experts,
    use_bf16: bool = False,
    **spmd_kwargs,
):
    """Full MoE forward. Returns (output, BassKernelResults)."""
    x = np.asarray(hidden_states, dtype=np.float32).reshape(-1, H)
    T = x.shape[0]

    idx_list, w_list = _route(top_experts, top_weights)
    maxc = max(len(i) for i in idx_list)
    n_pad = _pad_count(maxc)

    nc = _get_program(n_pad, use_bf16)

    in_maps = [
        _make_in_map(
            x, w_gate[e], w_up[e], w_down[e], idx_list[e], n_pad, use_bf16
        )
        for e in range(E)
    ]

    # Transient NRT exec failures (NRT_EXEC_UNIT_UNRECOVERABLE) have been
    # observed on the first 8-core execution of a fresh NEFF; retries clear
    # them.
    last_exc = None
    for attempt in range(3):
        try:
            res = run_bass_kernel_spmd(
                nc, in_maps, core_ids=list(range(E)), **spmd_kwargs
            )
            break
        except Exception as exc:
            last_exc = exc
            import time as _time

            _time.sleep(5)
    else:
        raise last_exc

    out = np.zeros((T, H), np.float32)
    for e in range(E):
        idx = idx_list[e]
        if len(idx) == 0:
            continue
        yt = res.results[e]["yt"]  # [hi, ho, t]
        y = yt.transpose(1, 0, 2).reshape(H, n_pad)[:, : len(idx)]  # [H, n_e]
        out[idx] += w_list[e][:, None] * y.T
    return out.reshape(np.asarray(hidden_states).shape).astype(np.float32), res


def kernel(hidden_states, top_weights, w_gate, w_up, w_down, top_experts):
    out, _ = run(hidden_states, top_weights, w_gate, w_up, w_down, top_experts)
    return out


# revision 27
# speedup vs baseline: 6.8537x; 1.4016x over previous
"""DBRX-experts MoE kernel for 8 Trainium2 NeuronCores (expert-parallel).

Strategy
--------
E=8 experts map 1:1 onto the 8 cores. The host gathers each expert's routed
tokens (top-k dispatch done in numpy — the "all-to-all" of the sharding hint
collapses to a host-side gather because kernel() already owns the full
inputs), pads them to a common count, and pre-lays-out the expert's weights
so the device kernel is a pure dense transposed MLP:

    G^T = Wg^T-tiles @ X^T      (contract H, out [F, T])
    U^T = Wu^T-tiles @ X^T
    Hmid^T = sigmoid(G^T) * G^T * U^T        (silu(g) * u)
    Y^T = Wd^T-tiles @ Hmid^T   (contract F, out [H, T])

All matmuls keep the weights stationary ([128,128] tiles) and the tokens
moving ([128, <=512]); tensors are declared float32r end-to-end so the PE
runs at 1 cycle/row (~2e-4 rel err vs fp32). The per-token combine weights
and the scatter-add back into the [T, H] output (the "all-reduce") are
applied on the host.

No device collectives are needed: cores are fully independent.
"""

import os

# The axon jax platform must stay visible even if the caller pinned cpu for
# its own reference computation (bass2jax needs jax.devices() -> axon).
if os.environ.get("JAX_PLATFORMS") == "cpu":
    os.environ["JAX_PLATFORMS"] = ""

import numpy as np

import concourse.bass as bass
import concourse.mybir as mybir
import concourse.tile as tile
from concourse.bass_utils import run_bass_kernel_spmd

E, H, F, P = 8, 2048, 2048, 128
HO, FO = H // P, F // P  # 16, 16

F32 = mybir.dt.float32
F32R = mybir.dt.float32r
BF16 = mybir.dt.bfloat16

_prog_cache: dict = {}


def _chunks_for(n_pad: int):
    """Split [0, n_pad) into equal chunks of <=512 (one PSUM bank of fp32).

    Chunks should be >=256 where possible so float32r matmuls run at full
    rate (1 cycle/row needs moving free dim >= 256).
    """
    n_ch = -(-n_pad // 512)
    assert n_pad % n_ch == 0
    cn = n_pad // n_ch
    return [(i * cn, cn) for i in range(n_ch)]


def _pad_count(maxc: int) -> int:
    """Smallest padded token count: multiple of 256 >= maxc (min 512), so
    equal chunking into <=512-wide pieces keeps every chunk >=256 wide."""
    n = max(512, -(-maxc // 256) * 256)
    # make equal chunks divide evenly: n is a multiple of 256 and
    # ceil(n/512) chunks of n/ceil divide n when n/256 splits evenly.
    while n % (-(-n // 512)) != 0 or (n // (-(-n // 512))) % 2 != 0:
        n += 256
    return n


def _legalize_sync_waits(nc):
    """Split sync waits exceeding the per-instruction ISA budget into NOPs.

    This walrus build rejects instructions with too many embedded sync-wait
    commands ("Too many sync wait commands", CoreV3GenImpl setupSyncWait):
    Matmult (fp32r, self-loading weights) tolerates 1, most opcodes 2, and
    Tile's scheduler freely emits more (e.g. the kernel-tail Drain). Moving
    the excess waits onto NoOp instructions placed immediately before the
    offender on the same engine queue is semantically identical: the engine
    blocks on the NOP first, then issues the original instruction.
    """
    ctr = 0
    for fn in nc.m.functions:
        for blk in fn.blocks:
            insts = blk.instructions
            out = []
            changed = False
            for inst in insts:
                si = inst.sync_info
                waits = list(si.on_wait) if si is not None and si.on_wait else []
                limit = 1
                if len(waits) > limit:
                    extra, keep = waits[:-limit], waits[-limit:]
                    for w in extra:
                        nop = mybir.InstNoOp(name=f"ant_sync_split_{ctr}", ins=[], outs=[])
                        ctr += 1
                        nop.engine = inst.engine
                        nop.sync_info = mybir.SyncInfo(on_wait=[w], on_update=[])
                        out.append(nop)
                    si.on_wait = keep
                    changed = True
                out.append(inst)
            if changed:
                blk.instructions = out


def _build_program(
    n_pad: int,
    use_bf16: bool,
    legalize: bool = True,
    reps: int = 1,
    wpool_bufs: int = 3,
    pp_bufs: int = 2,
    tpool_bufs: int = 3,
    gu_bufs: int | None = None,
    py_bufs: int | None = None,
    w_engine: str = "gpsimd",
):
    # fp32r operands must be *produced* as fp32r (BIR verifier: "consumed by
    # FP32r matmult but is not rounded to FP32r"), so the DRAM tensors and
    # every SBUF tile feeding a matmul are declared float32r end-to-end.
    dt_in = BF16 if use_bf16 else F32R
    chunks = _chunks_for(n_pad)

    nc = bass.Bass("TRN2")

    xt_d = nc.dram_tensor("xt", [P, HO, n_pad], dt_in, kind="ExternalInput")
    # gate and up interleaved on the second-to-last axis: one DMA per fo
    wgu_d = nc.dram_tensor("wgu", [FO, P, HO, 2, P], dt_in, kind="ExternalInput")
    wd_d = nc.dram_tensor("wd", [HO, P, FO, P], dt_in, kind="ExternalInput")
    yt_d = nc.dram_tensor("yt", [P, HO, n_pad], F32, kind="ExternalOutput")

    with tile.TileContext(nc) as tc:
        with (
            tc.tile_pool(name="xpool", bufs=1) as xpool,
            tc.tile_pool(name="wpool", bufs=wpool_bufs) as wpool,
            tc.tile_pool(name="hpool", bufs=1) as hpool,
            tc.tile_pool(name="tpool", bufs=tpool_bufs) as tpool,
            tc.tile_pool(name="pp", bufs=pp_bufs, space="PSUM") as pp,
        ):
            # X^T resident in SBUF: [hi, ho, t]; one DMA per ho-slab so the
            # first matmuls can start before the whole tensor lands.
            xt = xpool.tile([P, HO, n_pad], dt_in)
            for ho in range(HO):
                nc.sync.dma_start(xt[:, ho], xt_d[:, ho])

            for _rep in range(reps):
                _emit_mlp_body(
                    nc, n_pad, chunks, dt_in, xt,
                    wgu_d, wd_d, yt_d, wpool, hpool, tpool, pp,
                    gu_bufs=gu_bufs, py_bufs=py_bufs, w_engine=w_engine,
                )

    if legalize:
        _legalize_sync_waits(nc)
    return nc


def _emit_mlp_body(
    nc, n_pad, chunks, dt_in, xt, wgu_d, wd_d, yt_d, wpool, hpool, tpool, pp,
    gu_bufs=None, py_bufs=None, w_engine="gpsimd",
):
    w_dma = nc.gpsimd.dma_start if w_engine == "gpsimd" else nc.sync.dma_start
    if True:
        if True:
            # Hmid^T resident in SBUF: [fi, fo, t]
            hmid = hpool.tile([P, FO, n_pad], dt_in, tag="hmid")

            # Phase 1: G^T/U^T per 128-row slab of F, fused silu*up -> hmid
            for fo in range(FO):
                wgu_t = wpool.tile([P, HO, 2, P], dt_in, tag="wgu")
                if fo == 0:
                    # Split the very first weight load so the PE's first
                    # matmul only waits for a quarter of the tile (prologue
                    # shaving); steady-state loads stay whole-tile.
                    for q in range(4):
                        w_dma(
                            wgu_t[:, 4 * q : 4 * q + 4],
                            wgu_d[fo][:, 4 * q : 4 * q + 4],
                        )
                else:
                    w_dma(wgu_t[:], wgu_d[fo])
                for c0, cn in chunks:
                    pg = pp.tile([P, cn], F32, tag="pg", bufs=gu_bufs)
                    pu = pp.tile([P, cn], F32, tag="pu", bufs=gu_bufs)
                    for ho in range(HO):
                        nc.tensor.matmul(
                            pg,
                            wgu_t[:, ho, 0],
                            xt[:, ho, c0 : c0 + cn],
                            start=ho == 0,
                            stop=ho == HO - 1,
                        )
                    for ho in range(HO):
                        nc.tensor.matmul(
                            pu,
                            wgu_t[:, ho, 1],
                            xt[:, ho, c0 : c0 + cn],
                            start=ho == 0,
                            stop=ho == HO - 1,
                        )
                    # silu(g) * u = sigmoid(g) * g * u
                    sg = tpool.tile([P, cn], F32, tag="sg")
                    nc.scalar.activation(
                        sg, pg, mybir.ActivationFunctionType.Sigmoid
                    )
                    gs = tpool.tile([P, cn], F32, tag="gs")
                    nc.vector.tensor_mul(out=gs, in0=sg, in1=pg)
                    nc.vector.tensor_mul(
                        out=hmid[:, fo, c0 : c0 + cn], in0=gs, in1=pu
                    )

            # Phase 2: Y^T per 128-row slab of H
            for ho in range(HO):
                wd_t = wpool.tile([P, FO, P], dt_in, tag="wd")
                w_dma(wd_t[:], wd_d[ho])
                for c0, cn in chunks:
                    py = pp.tile([P, cn], F32, tag="py", bufs=py_bufs)
                    for fo in range(FO):
                        nc.tensor.matmul(
                            py,
                            wd_t[:, fo],
                            hmid[:, fo, c0 : c0 + cn],
                            start=fo == 0,
                            stop=fo == FO - 1,
                        )
                    yo = tpool.tile([P, cn], F32, tag="yo")
                    nc.vector.tensor_copy(out=yo, in_=py)
                    nc.sync.dma_start(yt_d[:, ho, c0 : c0 + cn], yo)


def _get_program(
    n_pad: int, use_bf16: bool, legalize: bool = True, reps: int = 1, **kw
):
    key = (n_pad, use_bf16, legalize, reps, tuple(sorted(kw.items())))
    if key not in _prog_cache:
        _prog_cache[key] = _build_program(n_pad, use_bf16, legalize, reps, **kw)
    return _prog_cache[key]


def _route(top_experts: np.ndarray, top_weights: np.ndarray):
    """Per-expert token indices and combined weights (duplicates merged)."""
    te = np.asarray(top_experts).astype(np.int64)
    tw = np.asarray(top_weights, dtype=np.float32)
    idx_list, w_list = [], []
    for e in range(E):
        m = te == e
        sel = m.any(axis=1)
        idx = np.nonzero(sel)[0]
        w = (tw * m).sum(axis=1)[idx].astype(np.float32)
        idx_list.append(idx)
        w_list.append(w)
    return idx_list, w_list


def _np_dt(use_bf16: bool):
    if use_bf16:
        import ml_dtypes

        return np.dtype(ml_dtypes.bfloat16)
    return np.dtype(np.float32)


def _make_in_map(x, w_gate_e, w_up_e, w_down_e, idx, n_pad, use_bf16):
    npdt = _np_dt(use_bf16)
    n_e = len(idx)
    xt = np.zeros((H, n_pad), np.float32)
    if n_e:
        xt[:, :n_e] = x[idx].T
    # [h, t] -> [hi, ho, t]
    xt_dev = np.ascontiguousarray(
        xt.reshape(HO, P, n_pad).transpose(1, 0, 2)
    ).astype(npdt)
    # Wg[f, h] -> [fo, hi, ho, fi]  (lhsT tiles [hi, fi] for each (fo, ho));
    # gate and up stacked on a new axis -> [fo, hi, ho, 2, fi]
    wg_dev = (
        np.asarray(w_gate_e, np.float32).reshape(FO, P, HO, P).transpose(0, 3, 2, 1)
    )
    wu_dev = (
        np.asarray(w_up_e, np.float32).reshape(FO, P, HO, P).transpose(0, 3, 2, 1)
    )
    wgu_dev = np.ascontiguousarray(
        np.stack([wg_dev, wu_dev], axis=3)
    ).astype(npdt)
    # Wd[h, f] -> [ho, fi, fo, hi]  (lhsT tiles [fi, hi] for each (ho, fo))
    wd_dev = np.ascontiguousarray(
        np.asarray(w_down_e, np.float32)
        .reshape(HO, P, FO, P)
        .transpose(0, 3, 2, 1)
    ).astype(npdt)
    return {"xt": xt_dev, "wgu": wgu_dev, "wd": wd_dev}


def run(
    hidden_states,
    top_weights,
    w_gate,
    w_up,
    w_down,
    top_experts,
    use_bf16: bool = False,
    **spmd_kwargs,
):
    """Full MoE forward. Returns (output, BassKernelResults)."""
    x = np.asarray(hidden_states, dtype=np.float32).reshape(-1, H)
    T = x.shape[0]

    idx_list, w_list = _route(top_experts, top_weights)
    maxc = max(len(i) for i in idx_list)
    n_pad = _pad_count(maxc)

    nc = _get_program(n_pad, use_bf16)

    in_maps = [
        _make_in_map(
            x, w_gate[e], w_up[e], w_down[e], idx_list[e], n_pad, use_bf16
        )
        for e in range(E)
    ]

    # Transient NRT exec failures (NRT_EXEC_UNIT_UNRECOVERABLE) have been
    # observed on the first 8-core execution of a fresh NEFF; retries clear
    # them.
    last_exc = None
    for attempt in range(3):
        try:
            res = run_bass_kernel_spmd(
                nc, in_maps, core_ids=list(range(E)), **spmd_kwargs
            )
            break
        except Exception as exc:
            last_exc = exc
            import time as _time

            _time.sleep(5)
    else:
        raise last_exc

    out = np.zeros((T, H), np.float32)
    for e in range(E):
        idx = idx_list[e]
        if len(idx) == 0:
            continue
        yt = res.results[e]["yt"]  # [hi, ho, t]
        y = yt.transpose(1, 0, 2).reshape(H, n_pad)[:, : len(idx)]  # [H, n_e]
        out[idx] += w_list[e][:, None] * y.T
    return out.reshape(np.asarray(hidden_states).shape).astype(np.float32), res


def kernel(hidden_states, top_weights, w_gate, w_up, w_down, top_experts):
    out, _ = run(hidden_states, top_weights, w_gate, w_up, w_down, top_experts)
    return out
